# revision 18
# baseline (speedup 1.0000x reference)
"""ExpertLinear (dense MoE blend) Trainium2 kernel — expert-sharded.

y[b,o] = sum_k ew[b,k] * (x[b,:] @ W[k,o,:]) + sum_k ew[b,k] * bias[k,o]

Sharding: one expert per core (E == 8 == NCORES). Each core computes its
expert's full GEMM z_c = x @ W[c].T for ALL B rows, scales by ew[:, c] on
eviction, and writes a bf16 partial; the host sums the 8 partials and adds
the (tiny) bias term. This reads each expert's weights exactly once
chip-wide: per-core HBM traffic is ~4 MB (vs ~18.5 MB for data-parallel).

Measured reality this schedule is tuned against (core-0 traces):
  - exec_time spans from the kernel's first instruction (gpsimd entry
    MEMSET) to the END of the runtime-appended teardown glue. The glue is
    NOT in the NEFF (walrus emits a 4-instruction tail); the runtime
    appends, per engine: DRAIN -> a FULL-barrier entry ring -> its share
    of a fixed ~250-semaphore wipe (Tensor's ~52 resets at ~115 ns are
    the largest/slowest share, ~6 us) -> exit ring -> NOTIFY. It cannot
    be shrunk, only overlapped/entered sooner.
  - All HWDGE input DMAs stripe over the SAME 16 chip queues, so arrival
    order == issue order and the stream is bandwidth-paced (~2.2-2.5
    TB/s chip-wide for 8 cores x 3 MB). Issuing chunks on other paths
    (scalar's ring group, SWDGE) makes them RACE the sync-issued stream
    for HBM and starves later chunks — keep every input on sync's FIFO
    (plus 0b on SWDGE, which is small and needed early). The matmul
    phase below is DMA-arrival-paced, not PE-paced, until ~i-tile 4; the
    measured floor for the first real matmul is ~5.1 us (issue ~0.6 +
    DGE start ~1.7 + transfer + ~2.3 us completion-receipt latency).
  - An idle PE re-throttles the HAM clock-gate (next ~7 matmuls run at
    ~2x cost): the N_DUMMY warmers must bridge boot -> chunk-0a landing
    with no gap, and chunk margins must prevent mid-phase stalls.

Layout/precision:
  - Host packs per-i-tile blocks [wT tile n | xT tile n] (bf16,
    contraction dim on partitions). I-tile 0 is split across the two DGE
    paths: 0a (HWDGE) = [wt0-h0 | full x tile | ew] feeds the first four
    matmuls; 0b (SWDGE, issued by gpsimd at engine boot) = wt0-h1 only,
    needed four matmuls later — outside SWDGE's slow (~3.5us) completion
    receipt. I-tiles 1-7 stream as 4 HWDGE chunks sized [1,1,2,3].
  - Exactly 8 HWDGE DMAs (6 in, yv + ya out), one per DMAHW sem lane, so
    no DMA carries a lane-recycle wait on top of its data wait (this
    walrus build rejects >1 sync wait per instruction). The same limit
    shapes the evict phase: ewt's bf16->f32 upconvert on DVE plus one
    tensor_scalar read-absorber and one ACT absorber keep every
    instruction at a single wait.
  - NO tile exit barrier at all (see _patch_drain_split): each engine
    falls straight from its last kernel instruction into the glue, whose
    own entry ring provides the ordering the barrier used to. The ring
    order (Tensor -> Scalar -> GpSimd -> Vector -> Sync wipe blocks)
    means Vector wipes the kernel sems only after Scalar's stream (last
    ACT evict + ya issue) retired, and Sync's output data waits are
    consumed before that. Output HBM-write receipts and any late sem
    increments complete under the glue / are re-zeroed by the next
    execution's entry clear.
  - PSUM: all 8 banks hold the [512, 1024] fp32 partial (4 b-chunks x 2
    o-halves). Accumulation is chunk-major/bank-major, with h1 BEFORE h0
    inside each (t, n) of the last chunk so ACT's (slower) evictions
    start one matmul earlier; banks complete staggered and the DVE/ACT
    evictions (x ew, ->bf16) pipeline behind the PE. yv ships via sync,
    ya via scalar right after its own evicts — every engine reaches the
    glue's entry ring ASAP after the last matmul.
"""

import numpy as np

B, E, IN, OUT = 512, 8, 1024, 1024
NCORES = 8
P = 128
NIT = IN // P      # 8 i-tiles (contraction chunks)
BT = B // P        # 4 b-chunks (output partition tiles)
NH = OUT // 512    # 2 o-halves (PSUM bank free-dim limit)
CW = OUT + B  # 1536 cols per i-tile block: wT tile (1024) + xT tile (512)
XOFF = OUT          # x region offset inside an i-tile block
# Warmers bridge engine-boot -> chunk-0a landing (~4.8-5.3 us: ~0.6 issue
# + ~1.1 DGE start + transfer + ~0.9 completion-receipt latency, with
# +-0.4 us of cross-core queue-race jitter). Full-partition warmers ramp
# the PE 630->427x5->361->216 ns (full clock after ~7); 8 of them end
# ~4.6 us, matching 0a's earlier landing once its issue is hoisted
# pre-barrier (see the block surgery at the end of _build).
N_DUMMY = 8
EWPAD = 16          # extra bf16 cols on chunk 0a carrying the ew column
A_XC = 512          # chunk 0a carries the FULL x tile: 4 h0 matmuls run
AW = 512 + A_XC + EWPAD   # before 0b (SWDGE, slow receipt) is needed
BW = 512                  # chunk 0b: [wt0 h1] only
# i-tile ranges per DMA chunk: fine-grained early chunks keep every
# chunk's completion semaphore ahead of the PE even when all 8 cores
# contend for HBM (a stall also re-throttles the HAM clock-gate, which
# costs 2-3 us extra — margins prevent it).
CHUNKS = [(0, 1), (1, 2), (2, 3), (3, 4), (4, 6), (6, 8)]

_compiled = None


def _patch_drain_split():
    """Suppress TileContext's kernel-tail teardown entirely:
    1) the walrus build in this container rejects any instruction carrying
       more than one sync wait, including the multi-wait Drain TileContext
       emits;
    2) the runtime-appended teardown glue (fixed ~250-sem wipe behind a
       full entry ring/barrier, ~6-7 us, measured inside exec_time) begins
       only after every engine retires — an exit barrier would only delay
       that. The glue's serialized wipe order means the kernel-sem range
       is wiped only after Scalar's stream retired, which is after all
       PSUM reads; sem increments landing after the wipe are re-zeroed by
       the next execution's entry clear."""
    import concourse.tile as tile_mod

    if getattr(tile_mod.TileContext, "_drain_split_patched", False):
        return

    def _drain_and_barrier(self, tick_clock, wait_clock):
        del tick_clock, wait_clock
        assert self.sems is not None
        popped = self.nc._tile_sem_poison_stack.pop()
        assert popped is self._sem_poison
        # bookkeeping of clear_and_free_semaphores WITHOUT emitting the
        # gpsimd clear + trailing barrier.
        sem_nums = [s.num for s in self.sems.allocated().values()]
        self.nc._state.prepend_free_semaphores(sem_nums)
        for poison_set in self.nc._tile_sem_poison_stack:
            poison_set.update(sem_nums)

    tile_mod.TileContext._drain_and_barrier = _drain_and_barrier
    tile_mod.TileContext._drain_split_patched = True


def _build():
    import concourse.bass as bass
    import concourse.mybir as mybir
    import concourse.tile as tile

    _patch_drain_split()

    f32 = mybir.dt.float32
    bf16 = mybir.dt.bfloat16
    Copy = mybir.ActivationFunctionType.Copy

    nc = bass.Bass()
    # chunk 0 split across the two DGE paths: 0a via HWDGE, 0b via SWDGE
    # (gpsimd issues it at engine-boot, and its DMASW sem lane is outside
    # the DMAHW budget)
    wx0a_d = nc.dram_tensor("wx0a", [P, AW], bf16, kind="ExternalInput")
    wx0b_d = nc.dram_tensor("wx0b", [P, BW], bf16, kind="ExternalInput")
    wxr_d = nc.dram_tensor(
        "wxr", [(NIT - 1) * P, CW], bf16, kind="ExternalInput"
    )
    yv_d = nc.dram_tensor("yv", [P, BT * 512], bf16, kind="ExternalOutput")
    ya_d = nc.dram_tensor("ya", [P, BT * 512], bf16, kind="ExternalOutput")

    with tile.TileContext(nc) as tc:
        with (
            tc.tile_pool(name="sb", bufs=1) as sb,
            tc.tile_pool(name="ps", bufs=1, space="PSUM") as psp,
        ):
            ewt = sb.tile([P, BT], f32, name="ewt", tag="ewt")
            scr_v = sb.tile([P, 1], f32, name="scrv", tag="scrv")
            scr_s = sb.tile([1, BT], f32, name="scrs", tag="scrs")
            wx0a = sb.tile([P, AW], bf16, name="wx0a", tag="wx0a")
            wx0b = sb.tile([P, BW], bf16, name="wx0b", tag="wx0b")
            wxs = [
                sb.tile([P, (e - s) * CW], bf16, name=f"wx{ci}", tag=f"wx{ci}")
                for ci, (s, e) in enumerate(CHUNKS[1:], start=1)
            ]
            y_v = sb.tile([P, BT * 512], bf16, name="yv", tag="yv")
            y_a = sb.tile([P, BT * 512], bf16, name="ya", tag="ya")
            pss = [
                [
                    psp.tile([P, 512], f32, name=f"ps{t}{h}", tag=f"ps{t}{h}")
                    for h in range(NH)
                ]
                for t in range(BT)
            ]

            # HAM warmers: FULL-PARTITION matmuls over (uninitialized)
            # y_v keep the whole PE array busy from engine-boot until the
            # first chunk lands. 1-row warmers only reach a mid pstate
            # (first real matmuls then run at 380-630 ns); a [128, 128]
            # lhsT activates all partitions so the clock-gate reaches 8/8.
            # Their garbage output fills bank (0,0), which the real
            # group's start=True clears.
            for _ in range(N_DUMMY):
                nc.tensor.matmul(
                    pss[0][0][:, :], y_v[:, 0:P], y_v[:, 0:512],
                    start=True, stop=True, skip_group_check=True,
                )

            # exactly 8 HWDGE DMAs in the whole kernel -> each DMAHW lane
            # is used once, so no DMA ever needs a lane-recycle wait on
            # top of its data wait (single-wait limit). wx0 first so the
            # PE's first real group is gated only by it; ALL inputs ride
            # sync's ring group: queue-FIFO order == consumption order,
            # and scalar's act ring group (measured ~1.5 us slower to
            # spin up) is reserved for the ya output at the end.
            nc.gpsimd.dma_start(wx0b[:], wx0b_d[:])
            nc.sync.dma_start(wx0a[:], wx0a_d[:])
            for ci, (s, e) in enumerate(CHUNKS[1:], start=1):
                src = wxr_d[(s - 1) * P:(e - 1) * P, :].rearrange(
                    "(n p) c -> p n c", p=P
                )
                dst = wxs[ci - 1][:].rearrange("p (n c) -> p n c", n=e - s)
                nc.sync.dma_start(dst, src)

            # i-tile 0: lhsT for all t and rhs h0 live in 0a; rhs h1 in
            # 0b. Order so the first four matmuls are gated only by 0a
            # and the first h1 matmul carries the single 0b wait.
            def _lhsT0(t):
                return wx0a[:, 512 + P * t:512 + P * (t + 1)]

            for t in range(BT):
                nc.tensor.matmul(
                    pss[t][0][:], _lhsT0(t), wx0a[:, 0:512],
                    start=True, stop=False,
                    skip_group_check=(t == 0),
                )
            for t in range(BT):
                nc.tensor.matmul(
                    pss[t][1][:], _lhsT0(t), wx0b[:, 0:512],
                    start=True, stop=False,
                )
            # remaining i-tiles: chunk-major so a group waits only on its
            # chunk's DMA; within a chunk, bank-major with h1 before h0
            # so in the last chunk ACT's stops lead DVE's and the
            # evictions pipeline behind the PE instead of serializing
            # after it.
            for ci, (s, e) in enumerate(CHUNKS[1:], start=1):
                wx = wxs[ci - 1]
                for t in range(BT):
                    for n in range(s, e):
                        off = (n - s) * CW
                        lhsT = wx[
                            :, off + XOFF + P * t:off + XOFF + P * (t + 1)
                        ]
                        for h in (1, 0):
                            nc.tensor.matmul(
                                pss[t][h][:], lhsT,
                                wx[:, off + 512 * h:off + 512 * (h + 1)],
                                start=False,
                                stop=(n == e - 1 and ci == len(CHUNKS) - 1),
                                skip_group_check=(t == 0 and h == 0),
                            )

            # ew rides in chunk 0 as bf16; DVE upconverts it once (this
            # also absorbs the chunk-0 DMA wait for DVE), and the ACT
            # absorber reads the converted copy so real evictions carry
            # only their PE wait (single-wait limit)
            nc.vector.tensor_copy(ewt[:], wx0a[:, 512 + A_XC:512 + A_XC + BT])
            # absorber: reads ewt through the tensor_scalar ptr path so the
            # real DVE evicts don't carry a second (DVE-seq) wait
            nc.vector.tensor_scalar_mul(scr_v[:], wx0a[:, 0:1], ewt[:, 0:1])
            nc.scalar.activation(scr_s[:], ewt[0:1, :], Copy)

            # evict: y[b,:] = ps[b,:] * ew[b]; DVE takes h=0, ACT h=1.
            for t in range(BT):
                sc = ewt[:, t:t + 1]
                nc.vector.tensor_scalar_mul(
                    y_v[:, t * 512:(t + 1) * 512], pss[t][0][:], sc
                )
                nc.scalar.activation(
                    y_a[:, t * 512:(t + 1) * 512], pss[t][1][:], Copy, scale=sc
                )
            # yv via sync (single DVE data wait), ya via scalar (no wait
            # at all in ACT program order): every engine reaches the
            # glue's entry ring ASAP. HBM-write receipts complete under
            # the glue.
            nc.sync.dma_start(yv_d[:], y_v[:])
            nc.scalar.dma_start(ya_d[:], y_a[:])

    # Hoist 0a's DMA issue (sync's first InstDMACopy: no waits, completion
    # = +16 on its DMAHW lane sem) from the tile body into the entry block
    # BEFORE sync's entry-barrier Drain: the doorbell then fires at ~0.2 us
    # instead of ~0.8-1.2 us, shifting the whole DGE-start + transfer +
    # sem-propagation pipeline earlier. Safe despite preceding the gpsimd
    # sem-clear MEMSETs: the transfer takes >=2.5 us, so the completion
    # increment cannot race the ~0.45 us clear; consumers wait on the lane
    # sem value, which is position-independent.
    f = list(nc.m.functions)[0]
    blocks = list(f.blocks)
    b0, b1 = blocks[0], blocks[1]
    l1 = list(b1.instructions)
    hoist = None
    for i, inst in enumerate(l1):
        if type(inst).__name__ == "InstDMACopy" and "SP" in str(inst.engine):
            hoist = l1.pop(i)
            break
    assert hoist is not None and not hoist.sync_info.on_wait
    b1.instructions = l1
    l0 = list(b0.instructions)
    for j, inst in enumerate(l0):
        if type(inst).__name__ == "InstDrain" and "SP" in str(inst.engine):
            break
    l0.insert(j, hoist)
    b0.instructions = l0

    return nc


def _get_compiled():
    global _compiled
    if _compiled is None:
        _compiled = _build()
    return _compiled


_pack_cache = None


def _make_in_maps(x, expert_weights, weight, bias):
    global _pack_cache
    import ml_dtypes

    bf16 = ml_dtypes.bfloat16
    if _pack_cache is None or _pack_cache[0] is not weight:
        w = np.asarray(weight, dtype=np.float32)
        wx0s, wxrs = [], []
        for c in range(NCORES):
            wT = w[c].T.reshape(NIT, P, OUT).astype(bf16)  # [p,o]=W[c,o,128n+p]
            a0 = np.zeros((P, AW), dtype=bf16)
            a0[:, :512] = wT[0, :, :512]
            b0 = np.ascontiguousarray(wT[0, :, 512:])
            ar = np.zeros((NIT - 1, P, CW), dtype=bf16)
            ar[:, :, :OUT] = wT[1:]
            wx0s.append((a0, b0))
            wxrs.append(ar)
        _pack_cache = (weight, wx0s, wxrs)
    _, wx0s, wxrs = _pack_cache

    x = np.asarray(x, dtype=np.float32)
    ew = np.asarray(expert_weights, dtype=np.float32)
    # xT tile n: [p, b] = x[b, 128n+p]
    xTb = x.T.reshape(NIT, P, B).astype(bf16)
    in_maps = []
    for c in range(NCORES):
        a0, b0 = wx0s[c]
        a0[:, 512:512 + A_XC] = xTb[0]
        a0[:, 512 + A_XC:512 + A_XC + BT] = (
            ew[:, c].reshape(BT, P).T.astype(bf16)
        )
        wxrs[c][:, :, XOFF:] = xTb[1:]
        in_maps.append({
            "wx0a": a0,
            "wx0b": b0,
            "wxr": wxrs[c].reshape((NIT - 1) * P, CW),
        })
    return in_maps


def kernel(x, expert_weights, weight, bias, _trace=False):
    from concourse.bass_utils import run_bass_kernel_spmd

    nc = _get_compiled()
    in_maps = _make_in_maps(x, expert_weights, weight, bias)
    res = run_bass_kernel_spmd(
        nc, in_maps, core_ids=list(range(NCORES)), trace=_trace
    )
    acc = np.zeros((B, OUT), dtype=np.float32)
    for r in res.results:
        # yv[p, t*512+j] = y[128t+p, j]; ya[p, t*512+j] = y[128t+p, 512+j]
        yv = np.asarray(r["yv"], dtype=np.float32).reshape(P, BT, 512)
        ya = np.asarray(r["ya"], dtype=np.float32).reshape(P, BT, 512)
        acc[:, :512] += yv.transpose(1, 0, 2).reshape(B, 512)
        acc[:, 512:] += ya.transpose(1, 0, 2).reshape(B, 512)
    ew = np.asarray(expert_weights, dtype=np.float32)
    b = np.asarray(bias, dtype=np.float32)
    y = acc + ew @ b
    if _trace:
        return y, res
    return y


# revision 19
# speedup vs baseline: 1.0393x; 1.0393x over previous
"""ExpertLinear (dense MoE blend) Trainium2 kernel — expert-sharded.

y[b,o] = sum_k ew[b,k] * (x[b,:] @ W[k,o,:]) + sum_k ew[b,k] * bias[k,o]

Sharding: one expert per core (E == 8 == NCORES). Each core computes its
expert's full GEMM z_c = x @ W[c].T for ALL B rows, scales by ew[:, c] on
eviction, and writes a bf16 partial; the host sums the 8 partials and adds
the (tiny) bias term. This reads each expert's weights exactly once
chip-wide: per-core HBM traffic is ~4 MB (vs ~18.5 MB for data-parallel).

Measured reality this schedule is tuned against (core-0 traces):
  - exec_time spans from the kernel's first instruction (gpsimd entry
    MEMSET) to the END of the runtime-appended teardown glue. The glue is
    NOT in the NEFF (walrus emits a 4-instruction tail); the runtime
    appends, per engine: DRAIN -> a FULL-barrier entry ring -> its share
    of a fixed ~250-semaphore wipe (Tensor's ~52 resets at ~115 ns are
    the largest/slowest share, ~6 us) -> exit ring -> NOTIFY. It cannot
    be shrunk, only overlapped/entered sooner.
  - All HWDGE input DMAs stripe over the SAME 16 chip queues, so arrival
    order == issue order and the stream is bandwidth-paced (~2.2-2.5
    TB/s chip-wide for 8 cores x 3 MB). Issuing chunks on other paths
    (scalar's ring group, SWDGE) makes them RACE the sync-issued stream
    for HBM and starves later chunks — keep every input on sync's FIFO
    (plus 0b on SWDGE, which is small and needed early). The matmul
    phase below is DMA-arrival-paced, not PE-paced, until ~i-tile 4; the
    measured floor for the first real matmul is ~5.1 us (issue ~0.6 +
    DGE start ~1.7 + transfer + ~2.3 us completion-receipt latency).
  - An idle PE re-throttles the HAM clock-gate (next ~7 matmuls run at
    ~2x cost): the N_DUMMY warmers must bridge boot -> chunk-0a landing
    with no gap, and chunk margins must prevent mid-phase stalls.

Layout/precision:
  - Host packs per-i-tile blocks [wT tile n | xT tile n] (bf16,
    contraction dim on partitions). I-tile 0 is split across the two DGE
    paths: 0a (HWDGE) = [wt0-h0 | full x tile | ew] feeds the first four
    matmuls; 0b (SWDGE, issued by gpsimd at engine boot) = wt0-h1 only,
    needed four matmuls later — outside SWDGE's slow (~3.5us) completion
    receipt. I-tiles 1-7 stream as 4 HWDGE chunks sized [1,1,2,3].
  - Exactly 8 HWDGE DMAs (6 in, yv + ya out), one per DMAHW sem lane, so
    no DMA carries a lane-recycle wait on top of its data wait (this
    walrus build rejects >1 sync wait per instruction). The same limit
    shapes the evict phase: ewt's bf16->f32 upconvert on DVE plus one
    tensor_scalar read-absorber and one ACT absorber keep every
    instruction at a single wait.
  - NO tile exit barrier at all (see _patch_drain_split): each engine
    falls straight from its last kernel instruction into the glue, whose
    own entry ring provides the ordering the barrier used to. The ring
    order (Tensor -> Scalar -> GpSimd -> Vector -> Sync wipe blocks)
    means Vector wipes the kernel sems only after Scalar's stream (last
    ACT evict + ya issue) retired, and Sync's output data waits are
    consumed before that. Output HBM-write receipts and any late sem
    increments complete under the glue / are re-zeroed by the next
    execution's entry clear.
  - PSUM: all 8 banks hold the [512, 1024] fp32 partial (4 b-chunks x 2
    o-halves). Accumulation is chunk-major/bank-major, with h1 BEFORE h0
    inside each (t, n) of the last chunk so ACT's (slower) evictions
    start one matmul earlier; banks complete staggered and the DVE/ACT
    evictions (x ew, ->bf16) pipeline behind the PE. yv ships via sync,
    ya via scalar right after its own evicts — every engine reaches the
    glue's entry ring ASAP after the last matmul.
"""

import numpy as np

B, E, IN, OUT = 512, 8, 1024, 1024
NCORES = 8
P = 128
NIT = IN // P      # 8 i-tiles (contraction chunks)
BT = B // P        # 4 b-chunks (output partition tiles)
NH = OUT // 512    # 2 o-halves (PSUM bank free-dim limit)
CW = OUT + B  # 1536 cols per i-tile block: wT tile (1024) + xT tile (512)
XOFF = OUT          # x region offset inside an i-tile block
# Warmers bridge engine-boot -> chunk-0a landing (~4.8-5.3 us: ~0.6 issue
# + ~1.1 DGE start + transfer + ~0.9 completion-receipt latency, with
# +-0.4 us of cross-core queue-race jitter). Full-partition warmers ramp
# the PE 630->427x5->361->216 ns (full clock after ~7); 8 of them end
# ~4.6 us, matching 0a's earlier landing once its issue is hoisted
# pre-barrier (see the block surgery at the end of _build).
N_DUMMY = 8
EWPAD = 16          # extra bf16 cols on chunk 0a carrying the ew column
A_XC = 512          # chunk 0a carries the FULL x tile: 4 h0 matmuls run
AW = 512 + A_XC + EWPAD   # before 0b (SWDGE, slow receipt) is needed
BW = 512                  # chunk 0b: [wt0 h1] only
# i-tile ranges per DMA chunk: fine-grained early chunks keep every
# chunk's completion semaphore ahead of the PE even when all 8 cores
# contend for HBM (a stall also re-throttles the HAM clock-gate, which
# costs 2-3 us extra — margins prevent it).
CHUNKS = [(0, 1), (1, 2), (2, 3), (3, 4), (4, 6), (6, 8)]

_compiled = None


def _patch_drain_split():
    """Suppress TileContext's kernel-tail teardown entirely:
    1) the walrus build in this container rejects any instruction carrying
       more than one sync wait, including the multi-wait Drain TileContext
       emits;
    2) the runtime-appended teardown glue (fixed ~250-sem wipe behind a
       full entry ring/barrier, ~6-7 us, measured inside exec_time) begins
       only after every engine retires — an exit barrier would only delay
       that. The glue's serialized wipe order means the kernel-sem range
       is wiped only after Scalar's stream retired, which is after all
       PSUM reads; sem increments landing after the wipe are re-zeroed by
       the next execution's entry clear."""
    import concourse.tile as tile_mod

    if getattr(tile_mod.TileContext, "_drain_split_patched", False):
        return

    def _drain_and_barrier(self, tick_clock, wait_clock):
        del tick_clock, wait_clock
        assert self.sems is not None
        popped = self.nc._tile_sem_poison_stack.pop()
        assert popped is self._sem_poison
        # bookkeeping of clear_and_free_semaphores WITHOUT emitting the
        # gpsimd clear + trailing barrier.
        sem_nums = [s.num for s in self.sems.allocated().values()]
        self.nc._state.prepend_free_semaphores(sem_nums)
        for poison_set in self.nc._tile_sem_poison_stack:
            poison_set.update(sem_nums)

    tile_mod.TileContext._drain_and_barrier = _drain_and_barrier
    tile_mod.TileContext._drain_split_patched = True


def _build():
    import concourse.bass as bass
    import concourse.mybir as mybir
    import concourse.tile as tile

    _patch_drain_split()

    f32 = mybir.dt.float32
    bf16 = mybir.dt.bfloat16
    Copy = mybir.ActivationFunctionType.Copy

    nc = bass.Bass()
    # chunk 0 split across the two DGE paths: 0a via HWDGE, 0b via SWDGE
    # (gpsimd issues it at engine-boot, and its DMASW sem lane is outside
    # the DMAHW budget)
    wx0a_d = nc.dram_tensor("wx0a", [P, AW], bf16, kind="ExternalInput")
    wx0b_d = nc.dram_tensor("wx0b", [P, BW], bf16, kind="ExternalInput")
    wxr_d = nc.dram_tensor(
        "wxr", [(NIT - 1) * P, CW], bf16, kind="ExternalInput"
    )
    yv_d = nc.dram_tensor("yv", [P, BT * 512], bf16, kind="ExternalOutput")
    ya_d = nc.dram_tensor("ya", [P, BT * 512], bf16, kind="ExternalOutput")

    with tile.TileContext(nc) as tc:
        with (
            tc.tile_pool(name="sb", bufs=1) as sb,
            tc.tile_pool(name="ps", bufs=1, space="PSUM") as psp,
        ):
            ewt = sb.tile([P, BT], f32, name="ewt", tag="ewt")
            scr_v = sb.tile([P, 1], f32, name="scrv", tag="scrv")
            scr_s = sb.tile([1, BT], f32, name="scrs", tag="scrs")
            wx0a = sb.tile([P, AW], bf16, name="wx0a", tag="wx0a")
            wx0b = sb.tile([P, BW], bf16, name="wx0b", tag="wx0b")
            wxs = [
                sb.tile([P, (e - s) * CW], bf16, name=f"wx{ci}", tag=f"wx{ci}")
                for ci, (s, e) in enumerate(CHUNKS[1:], start=1)
            ]
            y_v = sb.tile([P, BT * 512], bf16, name="yv", tag="yv")
            y_a = sb.tile([P, BT * 512], bf16, name="ya", tag="ya")
            pss = [
                [
                    psp.tile([P, 512], f32, name=f"ps{t}{h}", tag=f"ps{t}{h}")
                    for h in range(NH)
                ]
                for t in range(BT)
            ]

            # HAM warmers: FULL-PARTITION matmuls over (uninitialized)
            # y_v keep the whole PE array busy from engine-boot until the
            # first chunk lands. 1-row warmers only reach a mid pstate
            # (first real matmuls then run at 380-630 ns); a [128, 128]
            # lhsT activates all partitions so the clock-gate reaches 8/8.
            # Their garbage output fills bank (0,0), which the real
            # group's start=True clears.
            for _ in range(N_DUMMY):
                nc.tensor.matmul(
                    pss[0][0][:, :], y_v[:, 0:P], y_v[:, 0:512],
                    start=True, stop=True, skip_group_check=True,
                )

            # exactly 8 HWDGE DMAs in the whole kernel -> each DMAHW lane
            # is used once, so no DMA ever needs a lane-recycle wait on
            # top of its data wait (single-wait limit). wx0 first so the
            # PE's first real group is gated only by it; ALL inputs ride
            # sync's ring group: queue-FIFO order == consumption order,
            # and scalar's act ring group (measured ~1.5 us slower to
            # spin up) is reserved for the ya output at the end.
            nc.gpsimd.dma_start(wx0b[:], wx0b_d[:])
            nc.sync.dma_start(wx0a[:], wx0a_d[:])
            for ci, (s, e) in enumerate(CHUNKS[1:], start=1):
                src = wxr_d[(s - 1) * P:(e - 1) * P, :].rearrange(
                    "(n p) c -> p n c", p=P
                )
                dst = wxs[ci - 1][:].rearrange("p (n c) -> p n c", n=e - s)
                nc.sync.dma_start(dst, src)

            # i-tile 0: lhsT for all t and rhs h0 live in 0a; rhs h1 in
            # 0b. Order so the first four matmuls are gated only by 0a
            # and the first h1 matmul carries the single 0b wait.
            def _lhsT0(t):
                return wx0a[:, 512 + P * t:512 + P * (t + 1)]

            for t in range(BT):
                nc.tensor.matmul(
                    pss[t][0][:], _lhsT0(t), wx0a[:, 0:512],
                    start=True, stop=False,
                    skip_group_check=(t == 0),
                )

            def _i0h1():
                # i0-h1 consumed AFTER i-tile 1: 0b's SWDGE completion
                # receipt (3.5 us nominal, ~6.7 us on degraded device
                # state) gets 8 extra matmuls (~1.7 us) of margin. The
                # (t,1) banks are started by chunk 1's h1 matmuls, so
                # these accumulate.
                for t in range(BT):
                    nc.tensor.matmul(
                        pss[t][1][:], _lhsT0(t), wx0b[:, 0:512],
                        start=False, stop=False,
                    )

            # remaining i-tiles: chunk-major so a group waits only on its
            # chunk's DMA; within a chunk, bank-major with h1 before h0
            # so in the last chunk ACT's stops lead DVE's and the
            # evictions pipeline behind the PE instead of serializing
            # after it. Chunk 1's h1 matmuls START the (t,1) banks.
            for ci, (s, e) in enumerate(CHUNKS[1:], start=1):
                wx = wxs[ci - 1]
                for t in range(BT):
                    for n in range(s, e):
                        off = (n - s) * CW
                        lhsT = wx[
                            :, off + XOFF + P * t:off + XOFF + P * (t + 1)
                        ]
                        for h in (1, 0):
                            nc.tensor.matmul(
                                pss[t][h][:], lhsT,
                                wx[:, off + 512 * h:off + 512 * (h + 1)],
                                start=(ci == 1 and h == 1),
                                stop=(n == e - 1 and ci == len(CHUNKS) - 1),
                                skip_group_check=(t == 0 and h == 0),
                            )
                if ci == 1:
                    _i0h1()

            # ew rides in chunk 0 as bf16; DVE upconverts it once (this
            # also absorbs the chunk-0 DMA wait for DVE), and the ACT
            # absorber reads the converted copy so real evictions carry
            # only their PE wait (single-wait limit)
            nc.vector.tensor_copy(ewt[:], wx0a[:, 512 + A_XC:512 + A_XC + BT])
            # absorber: reads ewt through the tensor_scalar ptr path so the
            # real DVE evicts don't carry a second (DVE-seq) wait
            nc.vector.tensor_scalar_mul(scr_v[:], wx0a[:, 0:1], ewt[:, 0:1])
            nc.scalar.activation(scr_s[:], ewt[0:1, :], Copy)

            # evict: y[b,:] = ps[b,:] * ew[b]; DVE takes h=0, ACT h=1.
            for t in range(BT):
                sc = ewt[:, t:t + 1]
                nc.vector.tensor_scalar_mul(
                    y_v[:, t * 512:(t + 1) * 512], pss[t][0][:], sc
                )
                nc.scalar.activation(
                    y_a[:, t * 512:(t + 1) * 512], pss[t][1][:], Copy, scale=sc
                )
            # yv via sync (single DVE data wait), ya via scalar (no wait
            # at all in ACT program order): every engine reaches the
            # glue's entry ring ASAP. HBM-write receipts complete under
            # the glue.
            nc.sync.dma_start(yv_d[:], y_v[:])
            nc.scalar.dma_start(ya_d[:], y_a[:])

    # Hoist 0a's DMA issue (sync's first InstDMACopy: no waits, completion
    # = +16 on its DMAHW lane sem) from the tile body into the entry block
    # BEFORE sync's entry-barrier Drain: the doorbell then fires at ~0.2 us
    # instead of ~0.8-1.2 us, shifting the whole DGE-start + transfer +
    # sem-propagation pipeline earlier. Safe despite preceding the gpsimd
    # sem-clear MEMSETs: the transfer takes >=2.5 us, so the completion
    # increment cannot race the ~0.45 us clear; consumers wait on the lane
    # sem value, which is position-independent.
    f = list(nc.m.functions)[0]
    blocks = list(f.blocks)
    b0, b1 = blocks[0], blocks[1]
    l1 = list(b1.instructions)
    hoist = None
    for i, inst in enumerate(l1):
        if type(inst).__name__ == "InstDMACopy" and "SP" in str(inst.engine):
            hoist = l1.pop(i)
            break
    assert hoist is not None and not hoist.sync_info.on_wait
    b1.instructions = l1
    l0 = list(b0.instructions)
    for j, inst in enumerate(l0):
        if type(inst).__name__ == "InstDrain" and "SP" in str(inst.engine):
            break
    l0.insert(j, hoist)
    b0.instructions = l0

    return nc


def _get_compiled():
    global _compiled
    if _compiled is None:
        _compiled = _build()
    return _compiled


_pack_cache = None


def _make_in_maps(x, expert_weights, weight, bias):
    global _pack_cache
    import ml_dtypes

    bf16 = ml_dtypes.bfloat16
    if _pack_cache is None or _pack_cache[0] is not weight:
        w = np.asarray(weight, dtype=np.float32)
        wx0s, wxrs = [], []
        for c in range(NCORES):
            wT = w[c].T.reshape(NIT, P, OUT).astype(bf16)  # [p,o]=W[c,o,128n+p]
            a0 = np.zeros((P, AW), dtype=bf16)
            a0[:, :512] = wT[0, :, :512]
            b0 = np.ascontiguousarray(wT[0, :, 512:])
            ar = np.zeros((NIT - 1, P, CW), dtype=bf16)
            ar[:, :, :OUT] = wT[1:]
            wx0s.append((a0, b0))
            wxrs.append(ar)
        _pack_cache = (weight, wx0s, wxrs)
    _, wx0s, wxrs = _pack_cache

    x = np.asarray(x, dtype=np.float32)
    ew = np.asarray(expert_weights, dtype=np.float32)
    # xT tile n: [p, b] = x[b, 128n+p]
    xTb = x.T.reshape(NIT, P, B).astype(bf16)
    in_maps = []
    for c in range(NCORES):
        a0, b0 = wx0s[c]
        a0[:, 512:512 + A_XC] = xTb[0]
        a0[:, 512 + A_XC:512 + A_XC + BT] = (
            ew[:, c].reshape(BT, P).T.astype(bf16)
        )
        wxrs[c][:, :, XOFF:] = xTb[1:]
        in_maps.append({
            "wx0a": a0,
            "wx0b": b0,
            "wxr": wxrs[c].reshape((NIT - 1) * P, CW),
        })
    return in_maps


def kernel(x, expert_weights, weight, bias, _trace=False):
    from concourse.bass_utils import run_bass_kernel_spmd

    nc = _get_compiled()
    in_maps = _make_in_maps(x, expert_weights, weight, bias)
    res = run_bass_kernel_spmd(
        nc, in_maps, core_ids=list(range(NCORES)), trace=_trace
    )
    acc = np.zeros((B, OUT), dtype=np.float32)
    for r in res.results:
        # yv[p, t*512+j] = y[128t+p, j]; ya[p, t*512+j] = y[128t+p, 512+j]
        yv = np.asarray(r["yv"], dtype=np.float32).reshape(P, BT, 512)
        ya = np.asarray(r["ya"], dtype=np.float32).reshape(P, BT, 512)
        acc[:, :512] += yv.transpose(1, 0, 2).reshape(B, 512)
        acc[:, 512:] += ya.transpose(1, 0, 2).reshape(B, 512)
    ew = np.asarray(expert_weights, dtype=np.float32)
    b = np.asarray(bias, dtype=np.float32)
    y = acc + ew @ b
    if _trace:
        return y, res
    return y
